# revision 16
# baseline (speedup 1.0000x reference)
"""Trainium2 Bass kernel for nn_CvtNodeInitializer (gnn_message_passing).

Strategy (per the sharding hint: partition nodes, route edges by tail-node
owner, replicate the projection weight):
  - Host: filter edges whose tail is a CVT node (only those contribute),
    sort by tail, and route each edge's feature rows to the core owning
    the tail. CVT nodes are compacted and greedily packed into windows of
    <=128 nodes AND <=128 edge slots, so each window is one PSUM tile and
    one 128-slot edge tile. Non-CVT rows never touch the device: the host
    scatters the computed CVT rows into a copy of node_tokens.
  - Device (SPMD, identical program on 8 cores): per window, one fused
    matmul X^T-chunks @ [W_msg.T | a_eff] produces msg AND the per-edge
    logits (a_eff = attn @ W_msg folds the attention dot into the
    projection). exp on the scalar engine, one-hot(seg)*q on DVE, then a
    single f32r matmul segment-reduces agg = OH.T @ msg and den = OH.T @ 1.
    out_row = agg/den + shared_cvt. X and W travel as bf16 (halves HBM
    traffic; matmuls run at 1 cyc/row instead of f32's 4).
  - DMA is grouped G windows per transfer to amortize descriptor-gen cost.
"""

import sys

sys.path.insert(0, "/opt/trn_rl_repo")

import numpy as np
import ml_dtypes

N_NODES = 200000
N_EDGES = 200000
HID = 256
NCORES = 8
P = 128
G = 8           # windows per DMA group
WAUG = 258      # msg cols (256) + logit col (1) + pad

_PROGRAM_CACHE: dict = {}


def _build_program(W: int, repeats: int = 1):
    """Per-core Bass program. W = windows per core (multiple of G)."""
    import concourse.bacc as bacc
    import concourse.mybir as mybir
    import concourse.tile as tile

    f32 = mybir.dt.float32
    f32r = mybir.dt.float32r
    bf16 = mybir.dt.bfloat16
    i32 = mybir.dt.int32
    Alu = mybir.AluOpType
    Act = mybir.ActivationFunctionType

    assert W % G == 0
    WG = W // G

    nc = bacc.Bacc()
    xt = nc.declare_dram_parameter("xt", [WG, P, G * 4 * P], bf16,
                                   isOutput=False)
    sc = nc.declare_dram_parameter("sc", [P, W], f32, isOutput=False)
    wch = nc.declare_dram_parameter("wch", [P, 4 * WAUG], bf16, isOutput=False)
    out = nc.declare_dram_parameter("out", [WG, P, G * WAUG], bf16,
                                    isOutput=True)

    with tile.TileContext(nc) as tc:
        with (
            tc.tile_pool(name="const", bufs=1) as cpool,
            tc.tile_pool(name="x", bufs=3) as xpool,
            tc.tile_pool(name="og", bufs=2) as ogpool,
            tc.tile_pool(name="msg", bufs=3) as mpool,
            tc.tile_pool(name="small", bufs=6) as spool,
            tc.tile_pool(name="pmsg", bufs=4, space="PSUM") as pmpool,
            tc.tile_pool(name="pagg", bufs=3, space="PSUM") as papool,
        ):
            # --- one-time constants ---
            wtile = cpool.tile([P, 4 * WAUG], bf16)
            sctile = cpool.tile([P, W], f32)
            io_i = cpool.tile([P, P], i32)
            io_f = cpool.tile([P, P], f32)
            nc.sync.dma_start(out=wtile[:], in_=wch[:])
            nc.sync.dma_start(out=sctile[:], in_=sc[:])
            nc.gpsimd.iota(io_i[:], pattern=[[1, P]], base=0, channel_multiplier=0)
            nc.vector.tensor_copy(io_f[:], io_i[:])

            def stage_front(w, xg):
                """msg+logit matmul, exp, msg copy, one-hot — for window w."""
                k = w % G
                pm = pmpool.tile([P, WAUG], f32, tag="pm")
                for c in range(4):
                    nc.tensor.matmul(
                        pm[:, :],
                        lhsT=xg[:, (k * 4 + c) * P:(k * 4 + c + 1) * P],
                        rhs=wtile[:, c * WAUG:(c + 1) * WAUG],
                        start=(c == 0),
                        stop=(c == 3),
                    )
                qt = spool.tile([P, 1], f32, tag="qt")
                nc.scalar.activation(qt[:, 0:1], pm[:, 256:257], Act.Exp)
                # msgb cols 0:256 = msg; cols 256:258 = 1.0 so the agg
                # matmul also produces den in pa col 256
                msgb = mpool.tile([P, WAUG], bf16, tag="msgb")
                nc.scalar.activation(msgb[:, 0:HID], pm[:, 0:HID], Act.Copy)
                nc.gpsimd.memset(msgb[:, HID:WAUG], 1.0)
                oh = spool.tile([P, P], bf16, tag="oh")
                nc.gpsimd.tensor_scalar(
                    out=oh[:, :],
                    in0=io_f[:, :],
                    scalar1=sctile[:, w:w + 1],
                    scalar2=qt[:, 0:1],
                    op0=Alu.is_equal,
                    op1=Alu.mult,
                )
                return msgb, oh

            def stage_back(w, msgb, oh, og):
                """segment-reduce [agg | den] and stage for store."""
                k = w % G
                pa = papool.tile([P, WAUG], f32, tag="pa")
                nc.tensor.matmul(pa[:, :], lhsT=oh[:, :], rhs=msgb[:, :],
                                 start=True, stop=True)
                nc.vector.tensor_copy(og[:, k * WAUG:(k + 1) * WAUG], pa[:, :])

            def all_windows():
                prev = None  # (w, msgb, oh, og)
                for g in range(WG):
                    xg = xpool.tile([P, G * 4 * P], bf16, tag="xg")
                    og = ogpool.tile([P, G * WAUG], bf16, tag="og")
                    nc.sync.dma_start(out=xg[:], in_=xt[g])
                    for k in range(G):
                        w = g * G + k
                        front = stage_front(w, xg)
                        if prev is not None:
                            stage_back(*prev)
                            if prev[0] % G == G - 1:
                                pg = prev[3]
                                nc.sync.dma_start(out=out[prev[0] // G], in_=pg)
                        prev = (w, front[0], front[1], og)
                stage_back(*prev)
                nc.sync.dma_start(out=out[prev[0] // G], in_=prev[3])

            if repeats == 1:
                all_windows()
            else:
                with tc.For_i(0, repeats, 1) as _iv:
                    all_windows()

    nc.compile()
    return nc


def _host_prep(node_tokens, relation_tokens, edge_index, node_is_cvt,
               shared_cvt, attn_vector, W_msg, n_cores=NCORES):
    """Routing + per-core input construction.

    Returns (in_maps, W, scatter) where scatter = (node_ids, flat_rows)
    per core: out_full[node_ids] = dev_out[flat_rows]."""
    node_tokens = np.asarray(node_tokens, np.float32)
    relation_tokens = np.asarray(relation_tokens, np.float32)
    n_nodes, hid = node_tokens.shape

    tails = np.asarray(edge_index[1], dtype=np.int64)
    cvt = np.asarray(node_is_cvt, dtype=bool)
    cvt_nodes = np.nonzero(cvt)[0]                      # sorted CVT node ids
    ncvt = len(cvt_nodes)

    eids = np.nonzero(cvt[tails])[0]                    # contributing edges
    et = tails[eids]
    order = np.argsort(et, kind="stable")
    eids = eids[order]
    et = et[order]

    # per-CVT-node edge counts (aligned with cvt_nodes order)
    cnt_per_node = np.bincount(et, minlength=n_nodes)[cvt_nodes]
    assert cnt_per_node.max() <= P, "node with >128 edges unsupported"

    # split CVT nodes into 8 contiguous equal chunks
    bounds = [round(ncvt * c / n_cores) for c in range(n_cores + 1)]

    # greedy-pack each core's nodes into windows (<=128 nodes, <=128 edges)
    win = np.empty(ncvt, np.int64)
    seg = np.empty(ncvt, np.int64)
    estart = np.empty(ncvt, np.int64)
    Ws = []
    for c in range(n_cores):
        lo, hi = bounds[c], bounds[c + 1]
        w = 0
        nodes_in = 0
        edges_in = 0
        for i in range(lo, hi):
            k = cnt_per_node[i]
            if nodes_in == P or edges_in + k > P:
                w += 1
                nodes_in = 0
                edges_in = 0
            win[i] = w
            seg[i] = nodes_in
            estart[i] = edges_in
            nodes_in += 1
            edges_in += k
        Ws.append(w + 1 if hi > lo else 0)
    W = max(1, max(Ws))
    W = ((W + G - 1) // G) * G
    WG = W // G

    # per-edge window/slot (edges are sorted by tail; node rank via cumsum)
    node_rank_of_edge = np.searchsorted(cvt_nodes, et)   # index into cvt arrays
    first_edge_of_node = np.concatenate(
        [[0], np.cumsum(cnt_per_node)[:-1]]
    )
    rank_in_node = np.arange(len(et)) - first_edge_of_node[node_rank_of_edge]
    e_win = win[node_rank_of_edge]
    e_slot = estart[node_rank_of_edge] + rank_in_node
    core_of_node = np.searchsorted(bounds, np.arange(ncvt), side="right") - 1
    e_core = core_of_node[node_rank_of_edge]

    # edge features, routed: Xe_pad[core, w, slot] = [rel[e] | nod[e]]
    X = np.concatenate(
        [relation_tokens[eids], node_tokens[eids]], axis=1
    )                                                    # [ne, 2H] f32
    Xe = np.zeros((n_cores, W, P, 2 * hid), np.float32)
    Xe[e_core, e_win, e_slot] = X
    # xt[core, w, p, c*128+j] = Xe[core, w, j, c*128+p]; then group by G
    xt_all = (
        Xe.reshape(n_cores, W, P, 4, P)
        .transpose(0, 1, 4, 3, 2)
        .reshape(n_cores, WG, G, P, 4 * P)
        .transpose(0, 1, 3, 2, 4)
        .reshape(n_cores, WG, P, G * 4 * P)
        .astype(ml_dtypes.bfloat16)
    )

    # seg map: sc[core, slot, w] = local node id of edge in that slot
    segf = np.full((n_cores, W, P), -1000.0, np.float32)
    segf[e_core, e_win, e_slot] = seg[node_rank_of_edge].astype(np.float32)
    sc_all = np.ascontiguousarray(segf.transpose(0, 2, 1))

    # weights: wch[p, c*WAUG + h] = W_msg[h, c*128+p]; col 256 = a_eff
    a_eff = (attn_vector.astype(np.float64) @ np.asarray(W_msg, np.float64)
             ).astype(np.float32)                        # [2H]
    Wt = np.asarray(W_msg, np.float32).T                 # [2H, H]
    wch = np.zeros((P, 4 * WAUG), np.float32)
    for c in range(4):
        wch[:, c * WAUG:c * WAUG + hid] = Wt[c * P:(c + 1) * P, :]
        wch[:, c * WAUG + hid] = a_eff[c * P:(c + 1) * P]
    wch = wch.astype(ml_dtypes.bfloat16)

    in_maps = [
        {"xt": xt_all[c], "sc": sc_all[c], "wch": wch}
        for c in range(n_cores)
    ]
    # scatter: dev_out[core] reshaped [W*P, hid] row (w*P + seg) -> node id
    scatter = []
    for c in range(n_cores):
        lo, hi = bounds[c], bounds[c + 1]
        rows = win[lo:hi] * P + seg[lo:hi]
        scatter.append((cvt_nodes[lo:hi], rows))
    return in_maps, W, scatter


def kernel(**inputs) -> np.ndarray:
    from concourse import bass2jax

    node_tokens = np.asarray(inputs["node_tokens"], np.float32)
    in_maps, W, scatter = _host_prep(
        node_tokens,
        inputs["relation_tokens"],
        inputs["edge_index"],
        inputs["node_is_cvt"],
        inputs["shared_cvt"],
        inputs["attn_vector"],
        inputs["W_msg"],
    )
    nc = _PROGRAM_CACHE.get(W)
    if nc is None:
        nc = _build_program(W)
        _PROGRAM_CACHE[W] = nc
    results = bass2jax.run_bass_via_pjrt(nc, in_maps, n_cores=len(in_maps))
    hid = node_tokens.shape[1]
    shared = np.asarray(inputs["shared_cvt"], np.float32)
    out_full = node_tokens.copy()
    for c, r in enumerate(results):
        WG = r["out"].shape[0]
        dev = (np.asarray(r["out"]).astype(np.float32)
               .reshape(WG, P, G, WAUG)
               .transpose(0, 2, 1, 3).reshape(-1, WAUG))  # [W*P, WAUG]
        node_ids, rows = scatter[c]
        sel = dev[rows]
        agg = sel[:, :hid]
        den = np.maximum(sel[:, hid], 1e-30)[:, None]
        out_full[node_ids] = agg / den + shared
    return out_full


# revision 22
# speedup vs baseline: 2.8302x; 2.8302x over previous
"""Trainium2 Bass kernel for nn_CvtNodeInitializer (gnn_message_passing).

Strategy (per the sharding hint: partition nodes, route edges by tail-node
owner, replicate the projection weight):
  - Host: filter edges whose tail is a CVT node (only those contribute),
    sort by tail, and route each edge's feature rows to the core owning
    the tail. CVT nodes are compacted and greedily packed into windows of
    <=128 nodes AND <=128 edge slots, so each window is one PSUM tile and
    one 128-slot edge tile. Non-CVT rows never touch the device: the host
    scatters the computed CVT rows into a copy of node_tokens.
  - Device (SPMD, identical program on 8 cores): per window, one fused
    matmul X^T-chunks @ [W_msg.T | a_eff] produces msg AND the per-edge
    logits (a_eff = attn @ W_msg folds the attention dot into the
    projection). exp on the scalar engine, one-hot(seg)*q on DVE, then a
    single f32r matmul segment-reduces agg = OH.T @ msg and den = OH.T @ 1.
    out_row = agg/den + shared_cvt. X and W travel as bf16 (halves HBM
    traffic; matmuls run at 1 cyc/row instead of f32's 4).
  - DMA is grouped G windows per transfer to amortize descriptor-gen cost.
"""

import sys

sys.path.insert(0, "/opt/trn_rl_repo")

import numpy as np
import ml_dtypes

N_NODES = 200000
N_EDGES = 200000
HID = 256
NCORES = 8
P = 128
G = 8           # windows per DMA group
WAUG = 258      # msg cols (256) + logit col (1) + pad

_PROGRAM_CACHE: dict = {}


def _build_program(W: int, repeats: int = 1):
    """Per-core Bass program. W = windows per core (multiple of G)."""
    import concourse.bacc as bacc
    import concourse.mybir as mybir
    import concourse.tile as tile

    f32 = mybir.dt.float32
    f32r = mybir.dt.float32r
    bf16 = mybir.dt.bfloat16
    i32 = mybir.dt.int32
    Alu = mybir.AluOpType
    Act = mybir.ActivationFunctionType

    assert W % G == 0
    WG = W // G

    nc = bacc.Bacc()
    xt = nc.declare_dram_parameter("xt", [WG, P, G * 4 * P], bf16,
                                   isOutput=False)
    sc = nc.declare_dram_parameter("sc", [P, W], f32, isOutput=False)
    wch = nc.declare_dram_parameter("wch", [P, 4 * WAUG], bf16, isOutput=False)
    out = nc.declare_dram_parameter("out", [WG, P, G * WAUG], bf16,
                                    isOutput=True)

    with tile.TileContext(nc) as tc:
        with (
            tc.tile_pool(name="const", bufs=1) as cpool,
            tc.tile_pool(name="x", bufs=3) as xpool,
            tc.tile_pool(name="og", bufs=2) as ogpool,
            tc.tile_pool(name="msg", bufs=3) as mpool,
            tc.tile_pool(name="small", bufs=6) as spool,
            tc.tile_pool(name="pmsg", bufs=4, space="PSUM") as pmpool,
            tc.tile_pool(name="pagg", bufs=3, space="PSUM") as papool,
        ):
            # --- one-time constants ---
            wtile = cpool.tile([P, 4 * WAUG], bf16)
            sctile = cpool.tile([P, W], f32)
            io_i = cpool.tile([P, P], i32)
            io_b = cpool.tile([P, P], bf16)
            nc.sync.dma_start(out=wtile[:], in_=wch[:])
            nc.sync.dma_start(out=sctile[:], in_=sc[:])
            nc.gpsimd.iota(io_i[:], pattern=[[1, P]], base=0, channel_multiplier=0)
            nc.vector.tensor_copy(io_b[:], io_i[:])

            def stage_front(w, xg):
                """msg+logit matmul, exp, msg copy, one-hot — for window w."""
                k = w % G
                pm = pmpool.tile([P, WAUG], f32, tag="pm")
                for c in range(4):
                    nc.tensor.matmul(
                        pm[:, :],
                        lhsT=xg[:, (k * 4 + c) * P:(k * 4 + c + 1) * P],
                        rhs=wtile[:, c * WAUG:(c + 1) * WAUG],
                        start=(c == 0),
                        stop=(c == 3),
                    )
                qt = spool.tile([P, 1], f32, tag="qt")
                nc.scalar.activation(qt[:, 0:1], pm[:, 256:257], Act.Exp)
                # msgb cols 0:256 = msg; cols 256:258 = 1.0 so the agg
                # matmul also produces den in pa col 256
                msgb = mpool.tile([P, WAUG], bf16, tag="msgb")
                nc.scalar.activation(msgb[:, 0:HID], pm[:, 0:HID], Act.Copy)
                nc.gpsimd.memset(msgb[:, HID:WAUG], 1.0)
                oh = spool.tile([P, P], bf16, tag="oh")
                nc.vector.tensor_scalar(
                    out=oh[:, :],
                    in0=io_b[:, :],
                    scalar1=sctile[:, w:w + 1],
                    scalar2=qt[:, 0:1],
                    op0=Alu.is_equal,
                    op1=Alu.mult,
                )
                return msgb, oh

            def stage_back(w, msgb, oh, og):
                """segment-reduce [agg | den] and stage for store."""
                k = w % G
                pa = papool.tile([P, WAUG], f32, tag="pa")
                nc.tensor.matmul(pa[:, :], lhsT=oh[:, :], rhs=msgb[:, :],
                                 start=True, stop=True)
                nc.vector.tensor_copy(og[:, k * WAUG:(k + 1) * WAUG], pa[:, :])

            def all_windows():
                prev = None  # (w, msgb, oh, og)
                for g in range(WG):
                    xg = xpool.tile([P, G * 4 * P], bf16, tag="xg")
                    og = ogpool.tile([P, G * WAUG], bf16, tag="og")
                    nc.sync.dma_start(out=xg[:], in_=xt[g])
                    for k in range(G):
                        w = g * G + k
                        front = stage_front(w, xg)
                        if prev is not None:
                            stage_back(*prev)
                            if prev[0] % G == G - 1:
                                pg = prev[3]
                                nc.sync.dma_start(out=out[prev[0] // G], in_=pg)
                        prev = (w, front[0], front[1], og)
                stage_back(*prev)
                nc.sync.dma_start(out=out[prev[0] // G], in_=prev[3])

            if repeats == 1:
                all_windows()
            else:
                with tc.For_i(0, repeats, 1) as _iv:
                    all_windows()

    nc.compile()
    return nc


def _host_prep(node_tokens, relation_tokens, edge_index, node_is_cvt,
               shared_cvt, attn_vector, W_msg, n_cores=NCORES):
    """Routing + per-core input construction.

    Returns (in_maps, W, scatter) where scatter = (node_ids, flat_rows)
    per core: out_full[node_ids] = dev_out[flat_rows]."""
    node_tokens = np.asarray(node_tokens, np.float32)
    relation_tokens = np.asarray(relation_tokens, np.float32)
    n_nodes, hid = node_tokens.shape

    tails = np.asarray(edge_index[1], dtype=np.int64)
    cvt = np.asarray(node_is_cvt, dtype=bool)
    cvt_nodes = np.nonzero(cvt)[0]                      # sorted CVT node ids
    ncvt = len(cvt_nodes)

    eids = np.nonzero(cvt[tails])[0]                    # contributing edges
    et = tails[eids]
    order = np.argsort(et, kind="stable")
    eids = eids[order]
    et = et[order]

    # per-CVT-node edge counts (aligned with cvt_nodes order)
    cnt_per_node = np.bincount(et, minlength=n_nodes)[cvt_nodes]
    assert cnt_per_node.max() <= P, "node with >128 edges unsupported"

    # split CVT nodes into 8 contiguous equal chunks
    bounds = [round(ncvt * c / n_cores) for c in range(n_cores + 1)]

    # greedy-pack each core's nodes into windows (<=128 nodes, <=128 edges)
    win = np.empty(ncvt, np.int64)
    seg = np.empty(ncvt, np.int64)
    estart = np.empty(ncvt, np.int64)
    Ws = []
    for c in range(n_cores):
        lo, hi = bounds[c], bounds[c + 1]
        w = 0
        nodes_in = 0
        edges_in = 0
        for i in range(lo, hi):
            k = cnt_per_node[i]
            if nodes_in == P or edges_in + k > P:
                w += 1
                nodes_in = 0
                edges_in = 0
            win[i] = w
            seg[i] = nodes_in
            estart[i] = edges_in
            nodes_in += 1
            edges_in += k
        Ws.append(w + 1 if hi > lo else 0)
    W = max(1, max(Ws))
    W = ((W + G - 1) // G) * G
    WG = W // G

    # per-edge window/slot (edges are sorted by tail; node rank via cumsum)
    node_rank_of_edge = np.searchsorted(cvt_nodes, et)   # index into cvt arrays
    first_edge_of_node = np.concatenate(
        [[0], np.cumsum(cnt_per_node)[:-1]]
    )
    rank_in_node = np.arange(len(et)) - first_edge_of_node[node_rank_of_edge]
    e_win = win[node_rank_of_edge]
    e_slot = estart[node_rank_of_edge] + rank_in_node
    core_of_node = np.searchsorted(bounds, np.arange(ncvt), side="right") - 1
    e_core = core_of_node[node_rank_of_edge]

    # edge features, routed: Xe_pad[core, w, slot] = [rel[e] | nod[e]]
    X = np.concatenate(
        [relation_tokens[eids], node_tokens[eids]], axis=1
    )                                                    # [ne, 2H] f32
    Xe = np.zeros((n_cores, W, P, 2 * hid), np.float32)
    Xe[e_core, e_win, e_slot] = X
    # xt[core, w, p, c*128+j] = Xe[core, w, j, c*128+p]; then group by G
    xt_all = (
        Xe.reshape(n_cores, W, P, 4, P)
        .transpose(0, 1, 4, 3, 2)
        .reshape(n_cores, WG, G, P, 4 * P)
        .transpose(0, 1, 3, 2, 4)
        .reshape(n_cores, WG, P, G * 4 * P)
        .astype(ml_dtypes.bfloat16)
    )

    # seg map: sc[core, slot, w] = local node id of edge in that slot
    # (pad = -1.0: != any valid id 0..127 and exact in bf16)
    segf = np.full((n_cores, W, P), -1.0, np.float32)
    segf[e_core, e_win, e_slot] = seg[node_rank_of_edge].astype(np.float32)
    sc_all = np.ascontiguousarray(segf.transpose(0, 2, 1))

    # weights: wch[p, c*WAUG + h] = W_msg[h, c*128+p]; col 256 = a_eff
    a_eff = (attn_vector.astype(np.float64) @ np.asarray(W_msg, np.float64)
             ).astype(np.float32)                        # [2H]
    Wt = np.asarray(W_msg, np.float32).T                 # [2H, H]
    wch = np.zeros((P, 4 * WAUG), np.float32)
    for c in range(4):
        wch[:, c * WAUG:c * WAUG + hid] = Wt[c * P:(c + 1) * P, :]
        wch[:, c * WAUG + hid] = a_eff[c * P:(c + 1) * P]
    wch = wch.astype(ml_dtypes.bfloat16)

    in_maps = [
        {"xt": xt_all[c], "sc": sc_all[c], "wch": wch}
        for c in range(n_cores)
    ]
    # scatter: dev_out[core] reshaped [W*P, hid] row (w*P + seg) -> node id
    scatter = []
    for c in range(n_cores):
        lo, hi = bounds[c], bounds[c + 1]
        rows = win[lo:hi] * P + seg[lo:hi]
        scatter.append((cvt_nodes[lo:hi], rows))
    return in_maps, W, scatter


def kernel(**inputs) -> np.ndarray:
    from concourse import bass2jax

    node_tokens = np.asarray(inputs["node_tokens"], np.float32)
    in_maps, W, scatter = _host_prep(
        node_tokens,
        inputs["relation_tokens"],
        inputs["edge_index"],
        inputs["node_is_cvt"],
        inputs["shared_cvt"],
        inputs["attn_vector"],
        inputs["W_msg"],
    )
    nc = _PROGRAM_CACHE.get(W)
    if nc is None:
        nc = _build_program(W)
        _PROGRAM_CACHE[W] = nc
    results = bass2jax.run_bass_via_pjrt(nc, in_maps, n_cores=len(in_maps))
    hid = node_tokens.shape[1]
    shared = np.asarray(inputs["shared_cvt"], np.float32)
    out_full = node_tokens.copy()
    for c, r in enumerate(results):
        WG = r["out"].shape[0]
        dev = (np.asarray(r["out"]).astype(np.float32)
               .reshape(WG, P, G, WAUG)
               .transpose(0, 2, 1, 3).reshape(-1, WAUG))  # [W*P, WAUG]
        node_ids, rows = scatter[c]
        sel = dev[rows]
        agg = sel[:, :hid]
        den = np.maximum(sel[:, hid], 1e-30)[:, None]
        out_full[node_ids] = agg / den + shared
    return out_full
